# revision 16
# baseline (speedup 1.0000x reference)
"""Trainium2 Bass kernel: fused multi-head self-attention + output projection.

Problem (fixed shapes):
    N=2, S=2048, EMBED=1024, HEADS=16, HEAD_DIM=64, mask == all-ones.
    energy = einsum('nqhd,nkhd->nhqk', Q, K)
    attn   = softmax(energy / sqrt(EMBED), axis=k)
    out    = einsum('nhqk,nkhd->nqhd', attn, V).reshape(N,S,E) @ W_out.T + b_out

Sharding across 8 NeuronCores: core i handles batch n = i//4 and the 4 heads
[4g, 4g+4) with g = i%4 (data parallel over batch, tensor parallel over
heads).  Each core computes attention for its 4 heads plus the partial output
projection against the matching 256-row slice of W_out.T; the host sums the 4
partials per batch and adds b_out.

Device-side layout (everything stays transposed; no on-chip transposes, all
matmul operands bf16 — the only full-rate PE dtype; fp32r lowers to the
half-speed FP32-HIGH path):
    energyT[ki,qi] = matmul(lhsT=kT, rhs=qT)        (2 heads row-packed, ->PSUM f32)
    PT = exp(energyT/32)                            (ScalarE, 1024-wide, ->bf16)
    aoT[65,qi]    += matmul(lhsT=[v|1], rhs=PT)     (65th row = softmax denom)
    aonT = aoT[0:64] * bcast(1/aoT[64])             (DVE recip + GpSimd broadcast)
    proj[qi,e]    += matmul(lhsT=aonT, rhs=W'_h)    (accumulated over 4 heads)

The producer/consumer groups are software-pipelined (energy+exp of group g
emitted alongside the AV matmuls of group g-1, projection work drip-fed one
job per kc tick) so ScalarE — the 143us/core exp floor — never starves and
TensorE gaps stay under the ~3.4us HAM re-throttle window.
"""

import numpy as np

N, S, E, H, D = 2, 2048, 1024, 16, 64
P = 128                 # SBUF/PSUM partitions
QB = 512                # qi block width (PE moving-operand max for fp32)
KC = S // P             # 16 ki chunks of 128
NB = S // QB            # 4 qi blocks
HPC = 4                 # heads per core
SCALE = 1.0 / 32.0      # 1/sqrt(EMBED)

_PROGRAM = None


def _build_program():
    import concourse.bacc as bacc
    import concourse.mybir as mybir
    import concourse.tile as tile

    f32 = mybir.dt.float32
    bf16 = mybir.dt.bfloat16
    Exp = mybir.ActivationFunctionType.Exp

    nc = bacc.Bacc("TRN2", target_bir_lowering=False)

    qt_d = nc.dram_tensor("qt", [2, P, S], bf16, kind="ExternalInput")
    kt_d = nc.dram_tensor("kt", [2, P, S], bf16, kind="ExternalInput")
    v_d = nc.dram_tensor("v", [HPC, S, D], bf16, kind="ExternalInput")
    wt_d = nc.dram_tensor("wt", [HPC, D, E], bf16, kind="ExternalInput")
    out_d = nc.dram_tensor("out", [S, E], f32, kind="ExternalOutput")

    with tile.TileContext(nc) as tc:
        from contextlib import ExitStack

        with ExitStack() as ctx:
            singles = ctx.enter_context(tc.tile_pool(name="singles", bufs=1))
            ptp = ctx.enter_context(tc.tile_pool(name="ptp", bufs=24))
            rcp = ctx.enter_context(tc.tile_pool(name="rcp", bufs=4))
            bcp = ctx.enter_context(tc.tile_pool(name="bcp", bufs=3))
            outp = ctx.enter_context(tc.tile_pool(name="outp", bufs=3))
            epp = ctx.enter_context(tc.tile_pool(name="epp", bufs=2, space="PSUM"))
            aop = ctx.enter_context(tc.tile_pool(name="aop", bufs=3, space="PSUM"))
            ppp = ctx.enter_context(tc.tile_pool(name="ppp", bufs=1, space="PSUM"))

            # ---- persistent inputs -------------------------------------------------
            # one SBUF tensor per head for q/k, with head hh of pair p parked at
            # partitions [64*hh, 64*hh+64) (row-packed matmul pairs then stream
            # from distinct tensors, giving the XBUSes independent sources)
            qh = [singles.tile([P, S], bf16, tag=f"qh{i}", name=f"qh{i}") for i in range(4)]
            kh = [singles.tile([P, S], bf16, tag=f"kh{i}", name=f"kh{i}") for i in range(4)]
            # alternate DMA queues so the 8 input loads issue in parallel;
            # pair-0 tensors (needed by the first matmuls) go first
            for p in range(2):
                for hh in range(2):
                    i = 2 * p + hh
                    sl = slice(hh * D, (hh + 1) * D)
                    nc.sync.dma_start(out=kh[i][sl, :], in_=kt_d[p, sl])
                    nc.gpsimd.dma_start(out=qh[i][sl, :], in_=qt_d[p, sl])
            # v per head: [128, kc, 65] bf16, 65th column = 1.0 (denominator trick)
            # [v | 1] per head: column 64 = ones => aoT row 64 = softmax denom
            vt = [singles.tile([P, KC, D + 1], bf16, tag=f"vt{h}", name=f"vt{h}") for h in range(HPC)]
            for h in range(HPC):
                nc.gpsimd.dma_start(
                    out=vt[h][:, :, 0:D],
                    in_=v_d[h].rearrange("(c p) d -> p c d", p=P),
                )
                nc.vector.memset(vt[h][:, :, D : D + 1], 1.0)
            wt = [singles.tile([D, E], bf16, tag=f"wt{h}", name=f"wt{h}") for h in range(HPC)]
            for h in range(HPC):
                nc.gpsimd.dma_start(out=wt[h], in_=wt_d[h])
            # normalized attention outputs, transposed: per head [64, S]
            aont = [singles.tile([D, S], bf16, tag=f"aont{h}", name=f"aont{h}") for h in range(HPC)]
            # dummy exp: pulls the ACT table load into the DMA-wait window
            warm = singles.tile([1, 1], f32, tag="warm", name="warm")
            nc.vector.memset(warm, 0.0)
            nc.scalar.activation(warm, warm, Exp, scale=1.0)

            # ---- software-pipelined main loop --------------------------------------
            # groups: (qi block B, head pair p); produce (energy+exp) for group gi
            # while consuming (AV matmuls) group gi-1 so ScalarE never starves.
            groups = [(B, p) for B in range(NB) for p in range(2)]
            pts = {}  # gi -> list of 16 PT tiles
            proj_jobs = []  # deferred projection thunks, drip-fed into kc loops

            def emit_proj(Bc):
                for j in range(Bc * 4, Bc * 4 + 4):
                    ob = outp.tile([P, E], f32, tag="ob", name="ob")
                    for eb in range(2):

                        def mm_job(j=j, eb=eb, ob=ob):
                            pp = ppp.tile([P, QB], f32, tag="pp", name="pp")
                            for h in range(HPC):
                                nc.tensor.matmul(
                                    pp,
                                    lhsT=aont[h][:, j * P : (j + 1) * P],
                                    rhs=wt[h][:, eb * QB : (eb + 1) * QB],
                                    start=(h == 0),
                                    stop=(h == HPC - 1),
                                )
                            nc.vector.tensor_copy(ob[:, eb * QB : (eb + 1) * QB], pp)

                        proj_jobs.append(mm_job)

                    def dma_job(j=j, ob=ob):
                        nc.sync.dma_start(out=out_d[j * P : (j + 1) * P, :], in_=ob)

                    proj_jobs.append(dma_job)

            for gi in range(len(groups) + 1):
                prod = groups[gi] if gi < len(groups) else None
                cons = groups[gi - 1] if gi >= 1 else None
                if prod is not None:
                    pts[gi] = []
                if cons is not None:
                    ao = [aop.tile([D + 1, QB], f32, tag="ao", name="ao") for _ in range(2)]
                for kc in range(KC):
                    for _ in range(2 if prod is None else 1):
                        if proj_jobs:
                            proj_jobs.pop(0)()
                    if prod is not None:
                        B, p = prod
                        e = epp.tile([P, 2 * QB], f32, tag="ep", name="ep")
                        for hh in range(2):
                            i = 2 * p + hh
                            sl = slice(hh * D, (hh + 1) * D)
                            nc.tensor.matmul(
                                e[:, hh * QB : (hh + 1) * QB],
                                lhsT=kh[i][sl, kc * P : (kc + 1) * P],
                                rhs=qh[i][sl, B * QB : (B + 1) * QB],
                                start=True,
                                stop=True,
                            )
                        t = ptp.tile([P, 2 * QB], bf16, tag="pt", name="pt")
                        nc.scalar.activation(t, e, Exp, scale=SCALE)
                        pts[gi].append(t)
                    if cons is not None:
                        Bc, pc = cons
                        for hh in range(2):
                            nc.tensor.matmul(
                                ao[hh],
                                lhsT=vt[2 * pc + hh][:, kc, :],
                                rhs=pts[gi - 1][kc][:, hh * QB : (hh + 1) * QB],
                                start=(kc == 0),
                                stop=(kc == KC - 1),
                            )
                if cons is not None:
                    Bc, pc = cons
                    for hh in range(2):
                        h = 2 * pc + hh
                        # stage the denom row to SBUF partition 0: custom-DVE
                        # ops only address base partition 0 correctly on HW,
                        # and engine APs must start 32-aligned.
                        rc0 = rcp.tile([1, QB], f32, tag="rc0", name="rc0")
                        nc.vector.tensor_copy(rc0, ao[hh][D : D + 1, :])
                        rc = rcp.tile([1, QB], f32, tag="rc", name="rc")
                        nc.vector.reciprocal_approx_fast(out=rc, in_=rc0)
                        bc = bcp.tile([D, QB], f32, tag="bc", name="bc")
                        nc.gpsimd.partition_broadcast(bc, rc, channels=D)
                        nc.vector.tensor_mul(
                            aont[h][:, Bc * QB : (Bc + 1) * QB], ao[hh][0:D, :], bc
                        )
                    del pts[gi - 1]
                    if pc == 1:
                        # all 4 heads of qi block Bc are normalized: queue its
                        # projection, drip-fed into upcoming kc loops so it
                        # never blocks energy production (ScalarE supply).
                        emit_proj(Bc)
            for job in proj_jobs:
                job()

    nc.compile()
    return nc


def _program():
    global _PROGRAM
    if _PROGRAM is None:
        _PROGRAM = _build_program()
    return _PROGRAM


def _shard_inputs(values, keys, query, W_out):
    import ml_dtypes

    q = np.ascontiguousarray(np.asarray(query, np.float32)).reshape(N, S, H, D)
    k = np.ascontiguousarray(np.asarray(keys, np.float32)).reshape(N, S, H, D)
    v = np.ascontiguousarray(np.asarray(values, np.float32)).reshape(N, S, H, D)
    qT = np.ascontiguousarray(q.transpose(0, 2, 3, 1))  # [N, H, D, S]
    kT = np.ascontiguousarray(k.transpose(0, 2, 3, 1))
    vh = v.transpose(0, 2, 1, 3)  # [N, H, S, D] (view)
    WT = np.ascontiguousarray(np.asarray(W_out, np.float32).T)  # [E_in, E_out]

    in_maps = []
    for i in range(8):
        n, g = i // 4, i % 4
        h0 = 4 * g
        in_maps.append(
            {
                "qt": np.ascontiguousarray(qT[n, h0 : h0 + 4]).reshape(2, P, S).astype(ml_dtypes.bfloat16),
                "kt": np.ascontiguousarray(kT[n, h0 : h0 + 4]).reshape(2, P, S).astype(ml_dtypes.bfloat16),
                "v": np.ascontiguousarray(vh[n, h0 : h0 + 4]).astype(ml_dtypes.bfloat16),
                "wt": np.ascontiguousarray(WT[256 * g : 256 * (g + 1)]).reshape(HPC, D, E).astype(ml_dtypes.bfloat16),
            }
        )
    return in_maps


def kernel(values, keys, query, mask, W_out, b_out, _trace=False, _bkr_out=None):
    """Full inputs in, full output out.  mask is all-ones by construction and
    is ignored.  _trace/_bkr_out are test hooks (NTFF profiling)."""
    from concourse.bass_utils import run_bass_kernel_spmd

    nc = _program()
    in_maps = _shard_inputs(values, keys, query, W_out)
    bkr = run_bass_kernel_spmd(nc, in_maps, list(range(8)), trace=_trace)
    if _bkr_out is not None:
        _bkr_out.append(bkr)

    b = np.asarray(b_out, np.float32)
    out = np.empty((N, S, E), np.float32)
    for n in range(2):
        acc = bkr.results[4 * n]["out"].astype(np.float64)
        for j in range(1, 4):
            acc += bkr.results[4 * n + j]["out"]
        out[n] = (acc + b).astype(np.float32)
    return out


# revision 17
# speedup vs baseline: 1.0204x; 1.0204x over previous
"""Trainium2 Bass kernel: fused multi-head self-attention + output projection.

Problem (fixed shapes):
    N=2, S=2048, EMBED=1024, HEADS=16, HEAD_DIM=64, mask == all-ones.
    energy = einsum('nqhd,nkhd->nhqk', Q, K)
    attn   = softmax(energy / sqrt(EMBED), axis=k)
    out    = einsum('nhqk,nkhd->nqhd', attn, V).reshape(N,S,E) @ W_out.T + b_out

Sharding across 8 NeuronCores: core i handles batch n = i//4 and the 4 heads
[4g, 4g+4) with g = i%4 (data parallel over batch, tensor parallel over
heads).  Each core computes attention for its 4 heads plus the partial output
projection against the matching 256-row slice of W_out.T; the host sums the 4
partials per batch and adds b_out.

Device-side layout (everything stays transposed; no on-chip transposes, all
matmul operands bf16 — the only full-rate PE dtype; fp32r lowers to the
half-speed FP32-HIGH path):
    energyT[ki,qi] = matmul(lhsT=kT, rhs=qT)        (2 heads row-packed, ->PSUM f32)
    PT = exp(energyT/32)                            (ScalarE, 1024-wide, ->bf16)
    aoT[65,qi]    += matmul(lhsT=[v|1], rhs=PT)     (65th row = softmax denom)
    aonT = aoT[0:64] * bcast(1/aoT[64])             (DVE recip + GpSimd broadcast)
    proj[qi,e]    += matmul(lhsT=aonT, rhs=W'_h)    (accumulated over 4 heads)

The producer/consumer groups are software-pipelined (energy+exp of group g
emitted alongside the AV matmuls of group g-1, projection work drip-fed one
job per kc tick) so ScalarE — the 143us/core exp floor — never starves and
TensorE gaps stay under the ~3.4us HAM re-throttle window.
"""

import numpy as np

N, S, E, H, D = 2, 2048, 1024, 16, 64
P = 128                 # SBUF/PSUM partitions
QB = 512                # qi block width (PE moving-operand max for fp32)
KC = S // P             # 16 ki chunks of 128
NB = S // QB            # 4 qi blocks
HPC = 4                 # heads per core
SCALE = 1.0 / 32.0      # 1/sqrt(EMBED)

_PROGRAM = None


def _build_program():
    import concourse.bacc as bacc
    import concourse.mybir as mybir
    import concourse.tile as tile

    f32 = mybir.dt.float32
    bf16 = mybir.dt.bfloat16
    Exp = mybir.ActivationFunctionType.Exp

    nc = bacc.Bacc("TRN2", target_bir_lowering=False)

    qt_d = nc.dram_tensor("qt", [2, P, S], bf16, kind="ExternalInput")
    kt_d = nc.dram_tensor("kt", [2, P, S], bf16, kind="ExternalInput")
    v_d = nc.dram_tensor("v", [HPC, S, D], bf16, kind="ExternalInput")
    wt_d = nc.dram_tensor("wt", [HPC, D, E], bf16, kind="ExternalInput")
    out_d = nc.dram_tensor("out", [S, E], f32, kind="ExternalOutput")

    with tile.TileContext(nc) as tc:
        from contextlib import ExitStack

        with ExitStack() as ctx:
            singles = ctx.enter_context(tc.tile_pool(name="singles", bufs=1))
            ptp = ctx.enter_context(tc.tile_pool(name="ptp", bufs=24))
            rcp = ctx.enter_context(tc.tile_pool(name="rcp", bufs=4))
            bcp = ctx.enter_context(tc.tile_pool(name="bcp", bufs=3))
            outp = ctx.enter_context(tc.tile_pool(name="outp", bufs=3))
            epp = ctx.enter_context(tc.tile_pool(name="epp", bufs=2, space="PSUM"))
            aop = ctx.enter_context(tc.tile_pool(name="aop", bufs=3, space="PSUM"))
            ppp = ctx.enter_context(tc.tile_pool(name="ppp", bufs=1, space="PSUM"))

            # ---- persistent inputs -------------------------------------------------
            # one SBUF tensor per head for q/k, with head hh of pair p parked at
            # partitions [64*hh, 64*hh+64) (row-packed matmul pairs then stream
            # from distinct tensors, giving the XBUSes independent sources)
            qh = [singles.tile([P, S], bf16, tag=f"qh{i}", name=f"qh{i}") for i in range(4)]
            kh = [singles.tile([P, S], bf16, tag=f"kh{i}", name=f"kh{i}") for i in range(4)]
            # column-chunked loads on alternating DMA queues: the first energy
            # matmuls only need the first 512 columns of kh0/qh0, so chunking
            # lets PE start ~4x earlier than whole-tensor loads would
            for cc in range(4):
                cs = slice(cc * QB, (cc + 1) * QB)
                for p in range(2):
                    for hh in range(2):
                        i = 2 * p + hh
                        sl = slice(hh * D, (hh + 1) * D)
                        nc.sync.dma_start(out=kh[i][sl, cs], in_=kt_d[p, sl, cs])
                        nc.gpsimd.dma_start(out=qh[i][sl, cs], in_=qt_d[p, sl, cs])
            # v per head: [128, kc, 65] bf16, 65th column = 1.0 (denominator trick)
            # [v | 1] per head: column 64 = ones => aoT row 64 = softmax denom
            vt = [singles.tile([P, KC, D + 1], bf16, tag=f"vt{h}", name=f"vt{h}") for h in range(HPC)]
            for h in range(HPC):
                nc.gpsimd.dma_start(
                    out=vt[h][:, :, 0:D],
                    in_=v_d[h].rearrange("(c p) d -> p c d", p=P),
                )
                nc.vector.memset(vt[h][:, :, D : D + 1], 1.0)
            wt = [singles.tile([D, E], bf16, tag=f"wt{h}", name=f"wt{h}") for h in range(HPC)]
            for h in range(HPC):
                nc.gpsimd.dma_start(out=wt[h], in_=wt_d[h])
            # normalized attention outputs, transposed: per head [64, S]
            aont = [singles.tile([D, S], bf16, tag=f"aont{h}", name=f"aont{h}") for h in range(HPC)]
            # dummy exp: pulls the ACT table load into the DMA-wait window
            warm = singles.tile([1, 1], f32, tag="warm", name="warm")
            nc.vector.memset(warm, 0.0)
            nc.scalar.activation(warm, warm, Exp, scale=1.0)

            # ---- software-pipelined main loop --------------------------------------
            # groups: (qi block B, head pair p); produce (energy+exp) for group gi
            # while consuming (AV matmuls) group gi-1 so ScalarE never starves.
            groups = [(B, p) for B in range(NB) for p in range(2)]
            pts = {}  # gi -> list of 16 PT tiles
            proj_jobs = []  # deferred projection thunks, drip-fed into kc loops
            proj_cooldown = [0]  # ticks to wait before dripping fresh jobs

            def emit_proj(Bc):
                for j in range(Bc * 4, Bc * 4 + 4):
                    ob = outp.tile([P, E], f32, tag="ob", name="ob")
                    for eb in range(2):

                        def mm_job(j=j, eb=eb, ob=ob):
                            pp = ppp.tile([P, QB], f32, tag="pp", name="pp")
                            for h in range(HPC):
                                nc.tensor.matmul(
                                    pp,
                                    lhsT=aont[h][:, j * P : (j + 1) * P],
                                    rhs=wt[h][:, eb * QB : (eb + 1) * QB],
                                    start=(h == 0),
                                    stop=(h == HPC - 1),
                                )
                            nc.vector.tensor_copy(ob[:, eb * QB : (eb + 1) * QB], pp)

                        proj_jobs.append(mm_job)

                    def dma_job(j=j, ob=ob):
                        nc.sync.dma_start(out=out_d[j * P : (j + 1) * P, :], in_=ob)

                    proj_jobs.append(dma_job)

            for gi in range(len(groups) + 1):
                prod = groups[gi] if gi < len(groups) else None
                cons = groups[gi - 1] if gi >= 1 else None
                if prod is not None:
                    pts[gi] = []
                if cons is not None:
                    ao = [aop.tile([D + 1, QB], f32, tag="ao", name="ao") for _ in range(2)]
                for kc in range(KC):
                    if proj_cooldown[0] > 0:
                        proj_cooldown[0] -= 1
                    else:
                        for _ in range(2 if prod is None else 1):
                            if proj_jobs:
                                proj_jobs.pop(0)()
                    if prod is not None:
                        B, p = prod
                        e = epp.tile([P, 2 * QB], f32, tag="ep", name="ep")
                        for hh in range(2):
                            i = 2 * p + hh
                            sl = slice(hh * D, (hh + 1) * D)
                            nc.tensor.matmul(
                                e[:, hh * QB : (hh + 1) * QB],
                                lhsT=kh[i][sl, kc * P : (kc + 1) * P],
                                rhs=qh[i][sl, B * QB : (B + 1) * QB],
                                start=True,
                                stop=True,
                            )
                        t = ptp.tile([P, 2 * QB], bf16, tag="pt", name="pt")
                        nc.scalar.activation(t, e, Exp, scale=SCALE)
                        pts[gi].append(t)
                    if cons is not None:
                        Bc, pc = cons
                        for hh in range(2):
                            nc.tensor.matmul(
                                ao[hh],
                                lhsT=vt[2 * pc + hh][:, kc, :],
                                rhs=pts[gi - 1][kc][:, hh * QB : (hh + 1) * QB],
                                start=(kc == 0),
                                stop=(kc == KC - 1),
                            )
                if cons is not None:
                    Bc, pc = cons
                    for hh in range(2):
                        h = 2 * pc + hh
                        # stage the denom row to SBUF partition 0: custom-DVE
                        # ops only address base partition 0 correctly on HW,
                        # and engine APs must start 32-aligned.
                        rc0 = rcp.tile([1, QB], f32, tag="rc0", name="rc0")
                        nc.vector.tensor_copy(rc0, ao[hh][D : D + 1, :])
                        rc = rcp.tile([1, QB], f32, tag="rc", name="rc")
                        nc.vector.reciprocal_approx_fast(out=rc, in_=rc0)
                        bc = bcp.tile([D, QB], f32, tag="bc", name="bc")
                        nc.gpsimd.partition_broadcast(bc, rc, channels=D)
                        nc.vector.tensor_mul(
                            aont[h][:, Bc * QB : (Bc + 1) * QB], ao[hh][0:D, :], bc
                        )
                    del pts[gi - 1]
                    if pc == 1:
                        # all 4 heads of qi block Bc are normalized: queue its
                        # projection, drip-fed into upcoming kc loops so it
                        # never blocks energy production (ScalarE supply).
                        # cooldown: don't pop the first job until the aont
                        # writes have had time to land (in-order PE queue).
                        emit_proj(Bc)
                        proj_cooldown[0] = 4
            for job in proj_jobs:
                job()

    nc.compile()
    return nc


def _program():
    global _PROGRAM
    if _PROGRAM is None:
        _PROGRAM = _build_program()
    return _PROGRAM


def _shard_inputs(values, keys, query, W_out):
    import ml_dtypes

    q = np.ascontiguousarray(np.asarray(query, np.float32)).reshape(N, S, H, D)
    k = np.ascontiguousarray(np.asarray(keys, np.float32)).reshape(N, S, H, D)
    v = np.ascontiguousarray(np.asarray(values, np.float32)).reshape(N, S, H, D)
    qT = np.ascontiguousarray(q.transpose(0, 2, 3, 1))  # [N, H, D, S]
    kT = np.ascontiguousarray(k.transpose(0, 2, 3, 1))
    vh = v.transpose(0, 2, 1, 3)  # [N, H, S, D] (view)
    WT = np.ascontiguousarray(np.asarray(W_out, np.float32).T)  # [E_in, E_out]

    in_maps = []
    for i in range(8):
        n, g = i // 4, i % 4
        h0 = 4 * g
        in_maps.append(
            {
                "qt": np.ascontiguousarray(qT[n, h0 : h0 + 4]).reshape(2, P, S).astype(ml_dtypes.bfloat16),
                "kt": np.ascontiguousarray(kT[n, h0 : h0 + 4]).reshape(2, P, S).astype(ml_dtypes.bfloat16),
                "v": np.ascontiguousarray(vh[n, h0 : h0 + 4]).astype(ml_dtypes.bfloat16),
                "wt": np.ascontiguousarray(WT[256 * g : 256 * (g + 1)]).reshape(HPC, D, E).astype(ml_dtypes.bfloat16),
            }
        )
    return in_maps


def kernel(values, keys, query, mask, W_out, b_out, _trace=False, _bkr_out=None):
    """Full inputs in, full output out.  mask is all-ones by construction and
    is ignored.  _trace/_bkr_out are test hooks (NTFF profiling)."""
    from concourse.bass_utils import run_bass_kernel_spmd

    nc = _program()
    in_maps = _shard_inputs(values, keys, query, W_out)
    bkr = run_bass_kernel_spmd(nc, in_maps, list(range(8)), trace=_trace)
    if _bkr_out is not None:
        _bkr_out.append(bkr)

    b = np.asarray(b_out, np.float32)
    out = np.empty((N, S, E), np.float32)
    for n in range(2):
        acc = bkr.results[4 * n]["out"].astype(np.float64)
        for j in range(1, 4):
            acc += bkr.results[4 * n + j]["out"]
        out[n] = (acc + b).astype(np.float32)
    return out


# revision 18
# speedup vs baseline: 1.0324x; 1.0118x over previous
"""Trainium2 Bass kernel: fused multi-head self-attention + output projection.

Problem (fixed shapes):
    N=2, S=2048, EMBED=1024, HEADS=16, HEAD_DIM=64, mask == all-ones.
    energy = einsum('nqhd,nkhd->nhqk', Q, K)
    attn   = softmax(energy / sqrt(EMBED), axis=k)
    out    = einsum('nhqk,nkhd->nqhd', attn, V).reshape(N,S,E) @ W_out.T + b_out

Sharding across 8 NeuronCores: core i handles batch n = i//4 and the 4 heads
[4g, 4g+4) with g = i%4 (data parallel over batch, tensor parallel over
heads).  Each core computes attention for its 4 heads plus the partial output
projection against the matching 256-row slice of W_out.T; the host sums the 4
partials per batch and adds b_out.

Device-side layout (everything stays transposed; no on-chip transposes, all
matmul operands bf16 — the only full-rate PE dtype; fp32r lowers to the
half-speed FP32-HIGH path):
    energyT[ki,qi] = matmul(lhsT=kT, rhs=qT)        (2 heads row-packed, ->PSUM f32)
    PT = exp(energyT/32)                            (ScalarE, 1024-wide, ->bf16)
    aoT[65,qi]    += matmul(lhsT=[v|1], rhs=PT)     (65th row = softmax denom)
    aonT = aoT[0:64] * bcast(1/aoT[64])             (DVE recip + GpSimd broadcast)
    proj[qi,e]    += matmul(lhsT=aonT, rhs=W'_h)    (accumulated over 4 heads)

The producer/consumer groups are software-pipelined (energy+exp of group g
emitted alongside the AV matmuls of group g-1, projection work drip-fed one
job per kc tick) so ScalarE — the 143us/core exp floor — never starves and
TensorE gaps stay under the ~3.4us HAM re-throttle window.
"""

import numpy as np

N, S, E, H, D = 2, 2048, 1024, 16, 64
P = 128                 # SBUF/PSUM partitions
QB = 512                # qi block width (PE moving-operand max for fp32)
KC = S // P             # 16 ki chunks of 128
NB = S // QB            # 4 qi blocks
HPC = 4                 # heads per core
SCALE = 1.0 / 32.0      # 1/sqrt(EMBED)

_PROGRAM = None


def _build_program():
    import concourse.bacc as bacc
    import concourse.mybir as mybir
    import concourse.tile as tile

    f32 = mybir.dt.float32
    bf16 = mybir.dt.bfloat16
    Exp = mybir.ActivationFunctionType.Exp

    nc = bacc.Bacc("TRN2", target_bir_lowering=False)

    qt_d = nc.dram_tensor("qt", [2, P, S], bf16, kind="ExternalInput")
    kt_d = nc.dram_tensor("kt", [2, P, S], bf16, kind="ExternalInput")
    v_d = nc.dram_tensor("v", [HPC, S, D], bf16, kind="ExternalInput")
    wt_d = nc.dram_tensor("wt", [HPC, D, E], bf16, kind="ExternalInput")
    out_d = nc.dram_tensor("out", [S, E], f32, kind="ExternalOutput")

    with tile.TileContext(nc) as tc:
        from contextlib import ExitStack

        with ExitStack() as ctx:
            singles = ctx.enter_context(tc.tile_pool(name="singles", bufs=1))
            ptp = ctx.enter_context(tc.tile_pool(name="ptp", bufs=24))
            rcp = ctx.enter_context(tc.tile_pool(name="rcp", bufs=4))
            bcp = ctx.enter_context(tc.tile_pool(name="bcp", bufs=3))
            outp = ctx.enter_context(tc.tile_pool(name="outp", bufs=3))
            epp = ctx.enter_context(tc.tile_pool(name="epp", bufs=2, space="PSUM"))
            aop = ctx.enter_context(tc.tile_pool(name="aop", bufs=3, space="PSUM"))
            ppp = ctx.enter_context(tc.tile_pool(name="ppp", bufs=1, space="PSUM"))

            # ---- persistent inputs -------------------------------------------------
            # one SBUF tensor per head for q/k, with head hh of pair p parked at
            # partitions [64*hh, 64*hh+64) (row-packed matmul pairs then stream
            # from distinct tensors, giving the XBUSes independent sources)
            qh = [singles.tile([P, S], bf16, tag=f"qh{i}", name=f"qh{i}") for i in range(4)]
            kh = [singles.tile([P, S], bf16, tag=f"kh{i}", name=f"kh{i}") for i in range(4)]
            # loads ordered by first use: group 0 consumes all of kh0/kh1 (ki
            # axis) but only the first qi block of qh0/qh1; v is needed by the
            # first AV matmuls (~20us in); later qi blocks of q come last.
            def load_qk(i, cc, eng):
                p, hh = divmod(i, 2)
                cs = slice(cc * QB, (cc + 1) * QB)
                sl = slice(hh * D, (hh + 1) * D)
                eng.dma_start(out=kh[i][sl, cs] if eng is nc.sync else qh[i][sl, cs],
                              in_=(kt_d if eng is nc.sync else qt_d)[p, sl, cs])
            # v per head: [128, kc, 65] bf16, 65th column = 1.0 (denominator trick)
            # [v | 1] per head: column 64 = ones => aoT row 64 = softmax denom
            vt = [singles.tile([P, KC, D + 1], bf16, tag=f"vt{h}", name=f"vt{h}") for h in range(HPC)]
            wt = [singles.tile([D, E], bf16, tag=f"wt{h}", name=f"wt{h}") for h in range(HPC)]
            for cc in range(4):
                for i in range(2):
                    load_qk(i, cc, nc.sync)      # kh0/kh1, all ki chunks
            for i in range(2):
                load_qk(i, 0, nc.gpsimd)         # qh0/qh1 first qi block
            for h in range(HPC):
                nc.gpsimd.dma_start(
                    out=vt[h][:, :, 0:D],
                    in_=v_d[h].rearrange("(c p) d -> p c d", p=P),
                )
                nc.vector.memset(vt[h][:, :, D : D + 1], 1.0)
            for cc in range(4):
                for i in range(2, 4):
                    load_qk(i, cc, nc.sync)      # kh2/kh3
            for cc in range(1, 4):
                for i in range(2):
                    load_qk(i, cc, nc.gpsimd)    # qh0/qh1 remaining qi blocks
            for cc in range(4):
                for i in range(2, 4):
                    load_qk(i, cc, nc.gpsimd)    # qh2/qh3
            for h in range(HPC):
                nc.sync.dma_start(out=wt[h], in_=wt_d[h])
            # normalized attention outputs, transposed: per head [64, S]
            aont = [singles.tile([D, S], bf16, tag=f"aont{h}", name=f"aont{h}") for h in range(HPC)]
            # dummy exp: pulls the ACT table load into the DMA-wait window
            warm = singles.tile([1, 1], f32, tag="warm", name="warm")
            nc.vector.memset(warm, 0.0)
            nc.scalar.activation(warm, warm, Exp, scale=1.0)

            # ---- software-pipelined main loop --------------------------------------
            # groups: (qi block B, head pair p); produce (energy+exp) for group gi
            # while consuming (AV matmuls) group gi-1 so ScalarE never starves.
            groups = [(B, p) for B in range(NB) for p in range(2)]
            pts = {}  # gi -> list of 16 PT tiles
            proj_jobs = []  # deferred projection thunks, drip-fed into kc loops
            proj_cooldown = [0]  # ticks to wait before dripping fresh jobs

            def emit_proj(Bc):
                for j in range(Bc * 4, Bc * 4 + 4):
                    ob = outp.tile([P, E], f32, tag="ob", name="ob")
                    for eb in range(2):

                        def mm_job(j=j, eb=eb, ob=ob):
                            pp = ppp.tile([P, QB], f32, tag="pp", name="pp")
                            for h in range(HPC):
                                nc.tensor.matmul(
                                    pp,
                                    lhsT=aont[h][:, j * P : (j + 1) * P],
                                    rhs=wt[h][:, eb * QB : (eb + 1) * QB],
                                    start=(h == 0),
                                    stop=(h == HPC - 1),
                                )
                            nc.vector.tensor_copy(ob[:, eb * QB : (eb + 1) * QB], pp)

                        proj_jobs.append(mm_job)

                    def dma_job(j=j, ob=ob):
                        nc.sync.dma_start(out=out_d[j * P : (j + 1) * P, :], in_=ob)

                    proj_jobs.append(dma_job)

            for gi in range(len(groups) + 1):
                prod = groups[gi] if gi < len(groups) else None
                cons = groups[gi - 1] if gi >= 1 else None
                if prod is not None:
                    pts[gi] = []
                if cons is not None:
                    ao = [aop.tile([D + 1, QB], f32, tag="ao", name="ao") for _ in range(2)]
                for kc in range(KC):
                    if proj_cooldown[0] > 0:
                        proj_cooldown[0] -= 1
                    else:
                        for _ in range(2 if prod is None else 1):
                            if proj_jobs:
                                proj_jobs.pop(0)()
                    if prod is not None:
                        B, p = prod
                        e = epp.tile([P, 2 * QB], f32, tag="ep", name="ep")
                        for hh in range(2):
                            i = 2 * p + hh
                            sl = slice(hh * D, (hh + 1) * D)
                            nc.tensor.matmul(
                                e[:, hh * QB : (hh + 1) * QB],
                                lhsT=kh[i][sl, kc * P : (kc + 1) * P],
                                rhs=qh[i][sl, B * QB : (B + 1) * QB],
                                start=True,
                                stop=True,
                            )
                        t = ptp.tile([P, 2 * QB], bf16, tag="pt", name="pt")
                        nc.scalar.activation(t, e, Exp, scale=SCALE)
                        pts[gi].append(t)
                    if cons is not None:
                        Bc, pc = cons
                        for hh in range(2):
                            nc.tensor.matmul(
                                ao[hh],
                                lhsT=vt[2 * pc + hh][:, kc, :],
                                rhs=pts[gi - 1][kc][:, hh * QB : (hh + 1) * QB],
                                start=(kc == 0),
                                stop=(kc == KC - 1),
                            )
                if cons is not None:
                    Bc, pc = cons
                    for hh in range(2):
                        h = 2 * pc + hh
                        # stage the denom row to SBUF partition 0: custom-DVE
                        # ops only address base partition 0 correctly on HW,
                        # and engine APs must start 32-aligned.
                        rc0 = rcp.tile([1, QB], f32, tag="rc0", name="rc0")
                        nc.vector.tensor_copy(rc0, ao[hh][D : D + 1, :])
                        rc = rcp.tile([1, QB], f32, tag="rc", name="rc")
                        nc.vector.reciprocal_approx_fast(out=rc, in_=rc0)
                        bc = bcp.tile([D, QB], f32, tag="bc", name="bc")
                        nc.gpsimd.partition_broadcast(bc, rc, channels=D)
                        nc.vector.tensor_mul(
                            aont[h][:, Bc * QB : (Bc + 1) * QB], ao[hh][0:D, :], bc
                        )
                    del pts[gi - 1]
                    if pc == 1:
                        # all 4 heads of qi block Bc are normalized: queue its
                        # projection, drip-fed into upcoming kc loops so it
                        # never blocks energy production (ScalarE supply).
                        # cooldown: don't pop the first job until the aont
                        # writes have had time to land (in-order PE queue).
                        emit_proj(Bc)
                        proj_cooldown[0] = 4
            for job in proj_jobs:
                job()

    nc.compile()
    return nc


def _program():
    global _PROGRAM
    if _PROGRAM is None:
        _PROGRAM = _build_program()
    return _PROGRAM


def _shard_inputs(values, keys, query, W_out):
    import ml_dtypes

    q = np.ascontiguousarray(np.asarray(query, np.float32)).reshape(N, S, H, D)
    k = np.ascontiguousarray(np.asarray(keys, np.float32)).reshape(N, S, H, D)
    v = np.ascontiguousarray(np.asarray(values, np.float32)).reshape(N, S, H, D)
    qT = np.ascontiguousarray(q.transpose(0, 2, 3, 1))  # [N, H, D, S]
    kT = np.ascontiguousarray(k.transpose(0, 2, 3, 1))
    vh = v.transpose(0, 2, 1, 3)  # [N, H, S, D] (view)
    WT = np.ascontiguousarray(np.asarray(W_out, np.float32).T)  # [E_in, E_out]

    in_maps = []
    for i in range(8):
        n, g = i // 4, i % 4
        h0 = 4 * g
        in_maps.append(
            {
                "qt": np.ascontiguousarray(qT[n, h0 : h0 + 4]).reshape(2, P, S).astype(ml_dtypes.bfloat16),
                "kt": np.ascontiguousarray(kT[n, h0 : h0 + 4]).reshape(2, P, S).astype(ml_dtypes.bfloat16),
                "v": np.ascontiguousarray(vh[n, h0 : h0 + 4]).astype(ml_dtypes.bfloat16),
                "wt": np.ascontiguousarray(WT[256 * g : 256 * (g + 1)]).reshape(HPC, D, E).astype(ml_dtypes.bfloat16),
            }
        )
    return in_maps


def kernel(values, keys, query, mask, W_out, b_out, _trace=False, _bkr_out=None):
    """Full inputs in, full output out.  mask is all-ones by construction and
    is ignored.  _trace/_bkr_out are test hooks (NTFF profiling)."""
    from concourse.bass_utils import run_bass_kernel_spmd

    nc = _program()
    in_maps = _shard_inputs(values, keys, query, W_out)
    bkr = run_bass_kernel_spmd(nc, in_maps, list(range(8)), trace=_trace)
    if _bkr_out is not None:
        _bkr_out.append(bkr)

    b = np.asarray(b_out, np.float32)
    out = np.empty((N, S, E), np.float32)
    for n in range(2):
        acc = bkr.results[4 * n]["out"].astype(np.float64)
        for j in range(1, 4):
            acc += bkr.results[4 * n + j]["out"]
        out[n] = (acc + b).astype(np.float32)
    return out
